# revision 16
# baseline (speedup 1.0000x reference)
"""Boltzmann traffic-flow GNN on 8 Trainium2 NeuronCores.

Strategy (matches the sharding hint): nodes are sharded across the 8 cores
(1250 each); edges are partitioned by destination shard. Per message-passing
hop, each core writes its node-feature shard to DRAM, an AllGather collective
replicates the full feature table, and a SWDGE dma_gather pulls the per-edge
source rows. The segment-sum over incoming edges is done on the tensor engine
via per-chunk one-hot matmuls (128 edges x <=7 destination nodes per chunk).

To keep the program SPMD-uniform while per-core graph structure differs, each
core's local nodes are permuted (host-side bookkeeping): nodes are packed into
NCH groups of <=7 nodes whose incoming-edge count is <=128 (LPT balancing), so
chunk g always scatters into psum columns [7g, 7g+7). All data-dependent
structure lives in input tensors (gather indices, one-hot values); the Bass
program itself is fixed.
"""
import os
import sys
import time

sys.path.insert(0, "/opt/trn_rl_repo")

import numpy as np

import concourse.bacc as bacc
import concourse.tile as tile
from concourse import mybir
from concourse.bass_utils import run_bass_kernel_spmd

# problem constants (hardcoded per harness contract)
N = 10000
E = 160000
B = 4
T_IN = 12
D = 2
H = 128
Q = 15
HOPS = 2
OUT_LEN = int(os.environ.get("BTF_STEPS", "12"))
DT = 0.1
EPS = 1e-6

NCORES = 8
NPER = N // NCORES          # 1250 real nodes per core
GS = 7                      # nodes per chunk group
NCH = (NPER + GS - 1) // GS  # 179 chunks
NLOC = NCH * GS             # 1253 padded local node slots
NTAB = NCORES * NLOC        # feature table rows
F = B * Q                   # 60 packed feature width
ELEM = 64                   # table row = 64 fp32 = 256B (dma_gather needs %256B)
EPAD = NCH * 128            # gathered edges per core (padded)
P = 128
NG = (NLOC + P - 1) // P    # 10 node chunks of 128
# chunk starts; the last chunk overlaps the previous one so every chunk is a
# full 128 columns (overlap rows are computed twice with identical values)
CH0 = [i * P for i in range(NG - 1)] + [NLOC - P]
OVL = P * (NG - 1) - (NLOC - P)  # overlap rows in last chunk (27)
COLS = [(0, 512), (512, 1024), (1024, NLOC)]  # psum col tiles over local nodes
SUB_SPLIT = 4               # sub-gathers per event

f32 = mybir.dt.float32
f32r = mybir.dt.float32r
i16 = mybir.dt.int16

# use float32r (full-rate PE) for the wide-N dense matmuls; the one-hot
# segment-sum matmuls stay fp32 (N<=7 so fp32r wouldn't be faster anyway)
MM_WIDE_F32R = os.environ.get("BTF_F32R", "1") == "1"


def _wide(ap):
    return ap.bitcast(f32r) if MM_WIDE_F32R else ap


# --------------------------------------------------------------------------
# host preprocessing
# --------------------------------------------------------------------------

def _lpt_groups(degloc):
    """Assign NPER nodes to NCH groups of <=GS nodes, balancing summed degree.
    Returns group id per node. Raises if any group exceeds 128 edges."""
    import heapq
    order = np.argsort(-degloc, kind="stable")
    heap = [(0, 0, g) for g in range(NCH)]  # (edge_sum, node_cnt, g)
    heapq.heapify(heap)
    gid = np.empty(NPER, np.int64)
    pending = []
    for node in order:
        while True:
            s, cnt, g = heapq.heappop(heap)
            if cnt < GS:
                break
            pending.append((s, cnt, g))
        gid[node] = g
        heapq.heappush(heap, (s + int(degloc[node]), cnt + 1, g))
        for item in pending:
            heapq.heappush(heap, item)
        pending.clear()
    sums = np.bincount(gid, weights=degloc, minlength=NCH)
    if sums.max() > 128:
        raise RuntimeError(f"LPT group overflow: {sums.max()} edges")
    return gid


def _preprocess(inputs, src, dst, edge_w):
    src = np.asarray(src).astype(np.int64)
    dst = np.asarray(dst).astype(np.int64)
    edge_w = np.asarray(edge_w, np.float32)
    inputs = np.asarray(inputs, np.float32)

    deg = np.bincount(dst, minlength=N).astype(np.float32)
    norm = (np.clip(deg, 1.0, None) ** -0.5).astype(np.float32)
    wsum = np.bincount(dst, weights=edge_w.astype(np.float64), minlength=N)
    alpha = (edge_w / np.clip(wsum.astype(np.float32), EPS, None)[dst]).astype(np.float32)
    snv = (norm[src] * norm[dst]).astype(np.float32)

    table_row = np.empty(N, np.int64)   # global node -> table row
    cores = []
    for c in range(NCORES):
        n0 = c * NPER
        sel = np.nonzero((dst >= n0) & (dst < n0 + NPER))[0]
        dloc = dst[sel] - n0
        degloc = np.bincount(dloc, minlength=NPER)
        gid = _lpt_groups(degloc)
        # pi position: group g slot s -> 7g + s
        pi = np.empty(NPER, np.int64)
        slot = np.zeros(NCH, np.int64)
        for node in range(NPER):
            g = gid[node]
            pi[node] = g * GS + slot[g]
            slot[g] += 1
        table_row[n0:n0 + NPER] = c * NLOC + pi

        # edges sorted by destination pi position (stable)
        e_pi = pi[dloc]
        order = np.argsort(e_pi, kind="stable")
        es = sel[order]
        e_pi = e_pi[order]
        e_gid = e_pi // GS
        # slot within chunk = running index within group
        cnt = np.bincount(e_gid, minlength=NCH)
        starts = np.concatenate([[0], np.cumsum(cnt)[:-1]])
        within = np.arange(len(es)) - starts[e_gid]
        assert within.max(initial=0) < 128

        idx = np.zeros(EPAD, np.int64)                 # gather row per edge slot
        s_alpha = np.zeros((P, NCH * GS), np.float32)  # one-hot values
        s_norm = np.zeros((P, NCH * GS), np.float32)
        tok = e_gid * 128 + within                     # edge slot in [0, EPAD)
        idx[tok] = -1  # placeholder, fill below after table_row complete
        cores.append(dict(es=es, e_pi=e_pi, e_gid=e_gid, within=within, tok=tok,
                          idx=idx, s_alpha=s_alpha, s_norm=s_norm, pi=pi,
                          degloc=degloc))

    # second pass now that table_row is complete for all cores
    in_maps = []
    feat_tab = np.zeros((NTAB, ELEM), np.float32)
    last = inputs[:, -1]  # [B, N, D]
    # feature table rows: row = table_row[n], cols b*2+d
    featBD = np.transpose(last, (1, 0, 2)).reshape(N, B * D)  # [N, 8]
    feat_tab[table_row, :B * D] = featBD

    for c in range(NCORES):
        cd = cores[c]
        es, tok = cd["es"], cd["tok"]
        cd["idx"][:] = 0
        cd["idx"][tok] = table_row[src[es]]
        col = cd["e_gid"] * GS + (cd["e_pi"] - cd["e_gid"] * GS)  # = e_pi
        prow = cd["within"]
        cd["s_alpha"][prow, col] = alpha[es]
        cd["s_norm"][prow, col] = snv[es]
        # wrap idx to [128, EPAD//16] int16 (16-partition wrap, replicated x8)
        iw = cd["idx"].astype(np.int16).reshape(EPAD // 16, 16).T
        idx16 = np.tile(iw, (8, 1)).copy()
        # local features in pi order [8, NLOC]
        featloc = np.zeros((B * D, NLOC), np.float32)
        n0 = c * NPER
        featloc[:, cd["pi"]] = featBD[n0:n0 + NPER].T
        cd["idx16"] = idx16
        cd["featloc"] = featloc
    return dict(cores=cores, table_row=table_row, feat_tab=feat_tab,
                norm=norm, alpha=alpha)


# --------------------------------------------------------------------------
# device program
# --------------------------------------------------------------------------

def _sub_ranges():
    """(chunk_start, n_chunks) per sub-gather."""
    base = NCH // SUB_SPLIT
    rem = NCH % SUB_SPLIT
    out = []
    c0 = 0
    for g in range(SUB_SPLIT):
        n = base + (1 if g < rem else 0)
        out.append((c0, n))
        c0 += n
    return out


def _chunk_windows():
    """Per chunk: list of (tile_idx, psum_lo, psum_hi, rhs_off)."""
    wins = []
    for g in range(NCH):
        lo, hi = g * GS, g * GS + GS
        parts = []
        for t, (tlo, thi) in enumerate(COLS):
            a, b = max(lo, tlo), min(hi, thi)
            if a < b:
                parts.append((t, a - tlo, b - tlo, a - lo))
        wins.append(parts)
    return wins


def build_program():
    nc = bacc.Bacc(None, target_bir_lowering=False, debug=False)

    # external inputs
    ftab = nc.dram_tensor("feat_tab", [NTAB, ELEM], f32, kind="ExternalInput")
    gidx = nc.dram_tensor("gidx", [128, EPAD // 16], i16, kind="ExternalInput")
    sA = nc.dram_tensor("s_alpha", [P, NCH * GS], f32, kind="ExternalInput")
    sN = nc.dram_tensor("s_norm", [P, NCH * GS], f32, kind="ExternalInput")
    floc = nc.dram_tensor("featloc", [B * D, NLOC], f32, kind="ExternalInput")
    w1x = nc.dram_tensor("W1X", [B * D, (HOPS + 1) * B * H], f32, kind="ExternalInput")
    b1 = nc.dram_tensor("b1", [H, 1], f32, kind="ExternalInput")
    w2x = nc.dram_tensor("W2X", [H, (HOPS + 1) * B * F], f32, kind="ExternalInput")
    b2t = nc.dram_tensor("b2t", [F, 1], f32, kind="ExternalInput")
    wc1x = nc.dram_tensor("Wc1X", [F, B * H], f32, kind="ExternalInput")
    bc1 = nc.dram_tensor("bc1", [H, 1], f32, kind="ExternalInput")
    wc2x = nc.dram_tensor("Wc2X", [H, B * F], f32, kind="ExternalInput")
    bc2t = nc.dram_tensor("bc2t", [F, 1], f32, kind="ExternalInput")
    m3 = nc.dram_tensor("M3", [F, 3 * B], f32, kind="ExternalInput")
    idm = nc.dram_tensor("id128", [P, P], f32, kind="ExternalInput")

    # external outputs
    pred_out = nc.dram_tensor("pred_out", [OUT_LEN, NLOC, 8], f32, kind="ExternalOutput")
    vsq_out = nc.dram_tensor("vsq_out", [P, NG * B], f32, kind="ExternalOutput")

    # internal
    ag_in = nc.dram_tensor("ag_in", [NLOC, ELEM], f32)
    tabl = nc.dram_tensor("tabl", [NTAB, ELEM], f32, addr_space="Shared")

    subs = _sub_ranges()
    wins = _chunk_windows()
    import itertools
    _uid = itertools.count()

    with tile.TileContext(nc) as tc:
        with (
            tc.tile_pool(name="const", bufs=1) as cp,
            tc.tile_pool(name="sb", bufs=2) as sb,
            tc.tile_pool(name="gather", bufs=3) as gp,
            tc.tile_pool(name="persist", bufs=1) as pp,
            tc.tile_pool(name="ps_flow", bufs=3, space="PSUM") as ps_flow,
            tc.tile_pool(name="ps_big", bufs=2, space="PSUM") as ps_big,
            tc.tile_pool(name="ps_small", bufs=1, space="PSUM") as ps_small,
            tc.tile_pool(name="ps_mom", bufs=1, space="PSUM") as ps_mom,
            tc.tile_pool(name="ps_tp", bufs=1, space="PSUM") as ps_tp,
        ):
            # ---- load constants ----
            idx_sb = cp.tile([128, EPAD // 16], i16)
            nc.sync.dma_start(out=idx_sb[:], in_=gidx[:, :])
            sA_sb = cp.tile([P, NCH * GS], f32)
            nc.sync.dma_start(out=sA_sb[:], in_=sA[:, :])
            sN_sb = cp.tile([P, NCH * GS], f32)
            nc.sync.dma_start(out=sN_sb[:], in_=sN[:, :])
            w1_sb = cp.tile([B * D, (HOPS + 1) * B * H], f32)
            nc.sync.dma_start(out=w1_sb[:], in_=w1x[:, :])
            b1_sb = cp.tile([H, 1], f32)
            nc.sync.dma_start(out=b1_sb[:], in_=b1[:, :])
            w2_sb = cp.tile([H, (HOPS + 1) * B * F], f32)
            nc.sync.dma_start(out=w2_sb[:], in_=w2x[:, :])
            b2_sb = cp.tile([F, 1], f32)
            nc.sync.dma_start(out=b2_sb[:], in_=b2t[:, :])
            wc1_sb = cp.tile([F, B * H], f32)
            nc.sync.dma_start(out=wc1_sb[:], in_=wc1x[:, :])
            bc1_sb = cp.tile([H, 1], f32)
            nc.sync.dma_start(out=bc1_sb[:], in_=bc1[:, :])
            wc2_sb = cp.tile([H, B * F], f32)
            nc.sync.dma_start(out=wc2_sb[:], in_=wc2x[:, :])
            bc2_sb = cp.tile([F, 1], f32)
            nc.sync.dma_start(out=bc2_sb[:], in_=bc2t[:, :])
            m3_sb = cp.tile([F, 3 * B], f32)
            nc.sync.dma_start(out=m3_sb[:], in_=m3[:, :])
            id_sb = cp.tile([P, P], f32)
            nc.sync.dma_start(out=id_sb[:], in_=idm[:, :])

            vsq_acc = pp.tile([P, NG, B], f32)
            nc.vector.memset(vsq_acc[:], 0.0)

            # zero ag_in once (its cols 60:64 are never written per-step; the
            # sim's NaN-poisoned DRAM would otherwise trip require_finite)
            zt = sb.tile([P, NG, ELEM], f32, tag="zt")
            nc.vector.memset(zt[:], 0.0)
            nc.sync.dma_start(
                out=ag_in[0:P * (NG - 1), :].rearrange("(i p) e -> p i e", p=P),
                in_=zt[:, 0:NG - 1, :])
            nc.sync.dma_start(out=ag_in[NLOC - P:NLOC, :],
                              in_=zt[:, 0, :])

            # ---- helpers ----
            def gather_event(table):
                tiles = []
                for (c0, nch) in subs:
                    gt = gp.tile([P, nch, ELEM], f32, tag="gt")
                    n = nch * 128
                    nc.gpsimd.dma_gather(
                        out_ap=gt[:],
                        in_ap=table[:, :],
                        idxs_ap=idx_sb[:, c0 * 8:(c0 + nch) * 8],
                        num_idxs=n,
                        num_idxs_reg=n,
                        elem_size=ELEM,
                        single_packet=False,
                    )
                    tiles.append((gt, c0, nch))
                return tiles

            def spmm(tiles, s_sb, width, flow):
                first = [True, True, True]
                last_mm = {}
                for (gt, c0, nch) in tiles:
                    for cl in range(nch):
                        g = c0 + cl
                        for (t, plo, phi, roff) in wins[g]:
                            last_mm[t] = (g, plo)
                for (gt, c0, nch) in tiles:
                    for cl in range(nch):
                        g = c0 + cl
                        lhsT = gt[:, cl, 0:width]
                        for (t, plo, phi, roff) in wins[g]:
                            is_last = last_mm[t] == (g, plo)
                            nc.tensor.matmul(
                                out=flow[t][0:width, plo:phi],
                                lhsT=lhsT,
                                rhs=s_sb[:, g * GS + roff:g * GS + roff + (phi - plo)],
                                start=first[t],
                                stop=is_last,
                            )
                            first[t] = False

            def new_flow(width):
                return [ps_flow.tile([F, thi - tlo], f32, tag="flow",
                                     name=f"flow{i}_{next(_uid)}")
                        for i, (tlo, thi) in enumerate(COLS)]

            def write_table(src_sb, width, do_ag=True):
                """src_sb [F, NLOC] (rows 0:width used) -> transpose ->
                ag_in rows, then AllGather into tabl."""
                fts = sb.tile([P, NG, F], f32, tag="fts")
                for i in range(NG):
                    tp = ps_tp.tile([P, F], f32, tag="tp")
                    nc.tensor.transpose(
                        out=tp[:, 0:width],
                        in_=src_sb[0:width, CH0[i]:CH0[i] + P],
                        identity=id_sb[0:width, 0:width],
                    )
                    nc.vector.tensor_copy(out=fts[:, i, 0:width], in_=tp[:, 0:width])
                full = P * (NG - 1)
                nc.sync.dma_start(
                    out=ag_in[0:full, 0:width].rearrange("(i p) e -> p i e", p=P),
                    in_=fts[:, 0:NG - 1, 0:width],
                )
                nc.sync.dma_start(
                    out=ag_in[NLOC - P:NLOC, 0:width],
                    in_=fts[:, NG - 1, 0:width],
                )
                if do_ag:
                    nc.gpsimd.collective_compute(
                        "AllGather",
                        mybir.AluOpType.bypass,
                        ins=[ag_in[:, :]],
                        outs=[tabl[:, :]],
                        replica_groups=[list(range(NCORES))],
                    )

            # ================= encoder =================
            floc_sb = pp.tile([B * D, NLOC], f32)
            nc.sync.dma_start(out=floc_sb[:], in_=floc[:, :])

            def psum_to_rows(flow, dst_sb, width):
                for t, (tlo, thi) in enumerate(COLS):
                    nc.vector.tensor_copy(out=dst_sb[0:width, tlo:thi],
                                          in_=flow[t][0:width, 0:thi - tlo])

            # E0: h1 = A_norm @ feat
            tiles = gather_event(ftab)
            flow = new_flow(B * D)
            spmm(tiles, sN_sb, B * D, flow)
            h1 = pp.tile([B * D, NLOC], f32)
            psum_to_rows(flow, h1, B * D)
            write_table(h1, B * D)

            # E1: h2 = A_norm @ h1
            tiles = gather_event(tabl)
            flow = new_flow(B * D)
            spmm(tiles, sN_sb, B * D, flow)
            h2 = pp.tile([B * D, NLOC], f32)
            psum_to_rows(flow, h2, B * D)

            # layer1 dense: x_b = relu(sum_hop W1hop_b^T @ feat_hop + b1)
            # layer2 pre: u_k = sum_b W2X_kb^T @ x_b   (batch placement via
            # zero-masked weights so every operand starts at partition 0)
            u = [pp.tile([F, NLOC], f32, name=f"u{k}") for k in range(HOPS + 1)]
            hops_in = [floc_sb, h1, h2]
            for ci, (lo, hi) in enumerate(COLS):
                cw = hi - lo
                ups = []
                for k in range(HOPS + 1):
                    up = ps_flow.tile([F, 512], f32, tag="flow",
                                      name=f"up{k}_{next(_uid)}")
                    ups.append(up)
                for b in range(B):
                    xp = ps_big.tile([H, 512], f32, tag="big")
                    for a in range(HOPS + 1):
                        off = (a * B + b) * H
                        nc.tensor.matmul(out=xp[:, 0:cw],
                                         lhsT=_wide(w1_sb[:, off:off + H]),
                                         rhs=_wide(hops_in[a][:, lo:hi]),
                                         start=(a == 0), stop=(a == HOPS))
                    xb = sb.tile([H, 512], f32, tag="xb")
                    nc.scalar.activation(out=xb[:, 0:cw], in_=xp[:, 0:cw],
                                         func=mybir.ActivationFunctionType.Relu,
                                         bias=b1_sb[:])
                    for k in range(HOPS + 1):
                        off = (k * B + b) * F
                        nc.tensor.matmul(out=ups[k][:, 0:cw],
                                         lhsT=_wide(w2_sb[:, off:off + F]),
                                         rhs=_wide(xb[:, 0:cw]),
                                         start=(b == 0), stop=(b == B - 1))
                for k in range(HOPS + 1):
                    nc.vector.tensor_copy(out=u[k][:, lo:hi], in_=ups[k][:, 0:cw])

            # E2: v1 = A_norm @ u2 ; v1s = u1 + v1
            write_table(u[2], F)
            tiles = gather_event(tabl)
            flow = new_flow(F)
            spmm(tiles, sN_sb, F, flow)
            v1s = sb.tile([F, NLOC], f32, tag="v1s")
            for t, (tlo, thi) in enumerate(COLS):
                nc.vector.tensor_add(out=v1s[:, tlo:thi], in0=u[1][:, tlo:thi],
                                     in1=flow[t][:, 0:thi - tlo])

            # E3: v2 = A_norm @ v1s ; f0 = softplus(u0 + v2 + b2)
            write_table(v1s, F)
            tiles = gather_event(tabl)
            flow = new_flow(F)
            spmm(tiles, sN_sb, F, flow)
            # f0 = softplus(u0 + v2 + b2), softplus(x) = relu(x) + ln(1+exp(-|x|))
            f_cur = sb.tile([F, NLOC], f32, tag="f")
            AF = mybir.ActivationFunctionType
            for t, (tlo, thi) in enumerate(COLS):
                cw = thi - tlo
                tmp = sb.tile([F, 512], f32, tag="upd")
                nc.vector.tensor_add(out=tmp[:, 0:cw], in0=u[0][:, tlo:thi],
                                     in1=flow[t][:, 0:cw])
                x0 = sb.tile([F, 512], f32, tag="upd2")
                nc.scalar.activation(out=x0[:, 0:cw], in_=tmp[:, 0:cw],
                                     func=AF.Identity, bias=b2_sb[:])
                ax = sb.tile([F, 512], f32, tag="sp1")
                nc.scalar.activation(out=ax[:, 0:cw], in_=x0[:, 0:cw], func=AF.Abs)
                ex = sb.tile([F, 512], f32, tag="sp2")
                nc.scalar.activation(out=ex[:, 0:cw], in_=ax[:, 0:cw],
                                     func=AF.Exp, scale=-1.0)
                ln1 = sb.tile([F, 512], f32, tag="sp3")
                nc.scalar.activation(out=ln1[:, 0:cw], in_=ex[:, 0:cw],
                                     func=AF.Ln, bias=1.0)
                rx = sb.tile([F, 512], f32, tag="sp4")
                nc.scalar.activation(out=rx[:, 0:cw], in_=x0[:, 0:cw], func=AF.Relu)
                nc.vector.tensor_add(out=f_cur[:, tlo:thi], in0=rx[:, 0:cw],
                                     in1=ln1[:, 0:cw])

            # ================= time loop =================
            for t_step in range(OUT_LEN):
                write_table(f_cur, F)

                # omega = tanh(f @ Wc1 + bc1) @ Wc2 + bc2  (emitted first so PE
                # works on it while the gather DMA runs)
                omega = sb.tile([F, NLOC], f32, tag="omega")
                for (lo, hi) in COLS:
                    cw = hi - lo
                    o2 = ps_small.tile([F, 512], f32, tag="small")
                    for b in range(B):
                        o1 = ps_big.tile([H, 512], f32, tag="big")
                        nc.tensor.matmul(out=o1[:, 0:cw],
                                         lhsT=_wide(wc1_sb[:, H * b:H * b + H]),
                                         rhs=_wide(f_cur[:, lo:hi]),
                                         start=True, stop=True)
                        tb = sb.tile([H, 512], f32, tag="tanh")
                        nc.scalar.activation(out=tb[:, 0:cw], in_=o1[:, 0:cw],
                                             func=mybir.ActivationFunctionType.Tanh,
                                             bias=bc1_sb[:])
                        nc.tensor.matmul(out=o2[:, 0:cw],
                                         lhsT=_wide(wc2_sb[:, F * b:F * b + F]),
                                         rhs=_wide(tb[:, 0:cw]),
                                         start=(b == 0), stop=(b == B - 1))
                    nc.scalar.activation(out=omega[:, lo:hi], in_=o2[:, 0:cw],
                                         func=mybir.ActivationFunctionType.Identity,
                                         bias=bc2_sb[:])

                tiles = gather_event(tabl)
                flow = new_flow(F)
                spmm(tiles, sA_sb, F, flow)

                # f_next = relu(0.1*(flow + omega + 9*f))
                f_next = sb.tile([F, NLOC], f32, tag="f")
                for t, (tlo, thi) in enumerate(COLS):
                    cw = thi - tlo
                    t1 = sb.tile([F, 512], f32, tag="upd")
                    nc.vector.tensor_add(out=t1[:, 0:cw], in0=flow[t][:, 0:cw],
                                         in1=omega[:, tlo:thi])
                    t2 = sb.tile([F, 512], f32, tag="upd2")
                    nc.vector.scalar_tensor_tensor(
                        out=t2[:, 0:cw], in0=f_cur[:, tlo:thi], scalar=(1.0 - DT) / DT,
                        in1=t1[:, 0:cw], op0=mybir.AluOpType.mult,
                        op1=mybir.AluOpType.add)
                    nc.scalar.activation(out=f_next[:, tlo:thi], in_=t2[:, 0:cw],
                                         func=mybir.ActivationFunctionType.Relu,
                                         scale=DT)
                f_cur = f_next

                # moments: psumM[0:cn, 12i:12i+12] = f_chunk^T @ M3
                pm = ps_mom.tile([P, NG, 3 * B], f32, tag="mom")
                for i in range(NG):
                    nc.tensor.matmul(out=pm[:, i, :],
                                     lhsT=f_cur[:, CH0[i]:CH0[i] + P],
                                     rhs=m3_sb[:],
                                     start=(i == 0), stop=(i == NG - 1))
                dens = pm[:, :, :].rearrange("p i (b m) -> p i b m", m=3)[:, :, :, 0]
                m1v = pm[:, :, :].rearrange("p i (b m) -> p i b m", m=3)[:, :, :, 1]
                m2v = pm[:, :, :].rearrange("p i (b m) -> p i b m", m=3)[:, :, :, 2]
                cl = sb.tile([P, NG, B], f32, tag="m40a")
                nc.vector.tensor_scalar_max(out=cl[:], in0=dens, scalar1=EPS)
                inv = sb.tile([P, NG, B], f32, tag="m40b")
                nc.vector.reciprocal(out=inv[:], in_=cl[:])
                vel = sb.tile([P, NG, B], f32, tag="m40c")
                nc.vector.tensor_mul(out=vel[:], in0=m1v, in1=inv[:])
                m2i = sb.tile([P, NG, B], f32, tag="m40d")
                nc.vector.tensor_mul(out=m2i[:], in0=m2v, in1=inv[:])
                var = sb.tile([P, NG, B], f32, tag="m40e")
                nc.vector.tensor_mul(out=var[:], in0=vel[:], in1=vel[:])
                nc.vector.tensor_sub(out=var[:], in0=m2i[:], in1=var[:])
                sq = sb.tile([P, NG, B], f32, tag="m40f")
                nc.vector.tensor_mul(out=sq[:], in0=var[:], in1=var[:])
                nc.vector.tensor_add(out=vsq_acc[:], in0=vsq_acc[:], in1=sq[:])
                pred = sb.tile([P, NG, B, 2], f32, tag="pred")
                nc.vector.tensor_copy(out=pred[:, :, :, 0], in_=dens)
                nc.vector.tensor_copy(out=pred[:, :, :, 1], in_=vel[:])
                full = P * (NG - 1)
                nc.sync.dma_start(
                    out=pred_out[t_step, 0:full, :].rearrange("(i p) d -> p i d", p=P),
                    in_=pred[:, 0:NG - 1, :, :].rearrange("p i b d -> p i (b d)"),
                )
                nc.sync.dma_start(
                    out=pred_out[t_step, NLOC - P:NLOC, :],
                    in_=pred[:, NG - 1, :, :].rearrange("p b d -> p (b d)"),
                )

            nc.sync.dma_start(out=vsq_out[:, :],
                              in_=vsq_acc[:].rearrange("p i b -> p (i b)"))

    nc.compile()
    return nc


# --------------------------------------------------------------------------
# driver
# --------------------------------------------------------------------------

_CACHED = {}


def _get_program():
    if "nc" not in _CACHED:
        _CACHED["nc"] = build_program()
    return _CACHED["nc"]


def _make_runner(nc):
    """Reusable jitted SPMD runner (mirrors bass2jax.run_bass_via_pjrt but
    keeps the jitted callable so repeated timed invocations don't recompile)."""
    import jax
    from jax.experimental.shard_map import shard_map
    from jax.sharding import Mesh, PartitionSpec
    from concourse import bass2jax

    bass2jax.install_neuronx_cc_hook()
    partition_name = nc.partition_id_tensor.name if nc.partition_id_tensor else None

    in_names, out_names, out_avals, zero_outs = [], [], [], []
    for alloc in nc.m.functions[0].allocations:
        if not isinstance(alloc, mybir.MemoryLocationSet):
            continue
        name = alloc.memorylocations[0].name
        if alloc.kind == "ExternalInput":
            if name != partition_name:
                in_names.append(name)
        elif alloc.kind == "ExternalOutput":
            shape = tuple(alloc.tensor_shape)
            dtype = mybir.dt.np(alloc.dtype)
            out_names.append(name)
            out_avals.append(jax.core.ShapedArray(shape, dtype))
            zero_outs.append(np.zeros(shape, dtype))
    n_params = len(in_names)
    n_outs = len(out_avals)
    all_in_names = list(in_names) + list(out_names)
    if partition_name is not None:
        all_in_names.append(partition_name)
    donate = tuple(range(n_params, n_params + n_outs))

    def _body(*args):
        operands = list(args)
        if partition_name is not None:
            operands.append(bass2jax.partition_id_tensor())
        outs = bass2jax._bass_exec_p.bind(
            *operands,
            out_avals=tuple(out_avals),
            in_names=tuple(all_in_names),
            out_names=tuple(out_names),
            lowering_input_output_aliases=(),
            sim_require_finite=True,
            sim_require_nnan=True,
            nc=nc,
        )
        return tuple(outs)

    devices = jax.devices()[:NCORES]
    mesh = Mesh(np.asarray(devices), ("core",))
    in_specs = (PartitionSpec("core"),) * (n_params + n_outs)
    out_specs = (PartitionSpec("core"),) * n_outs
    sharded = jax.jit(
        shard_map(_body, mesh=mesh, in_specs=in_specs, out_specs=out_specs,
                  check_rep=False),
        donate_argnums=donate, keep_unused=True)

    def run(in_maps, timing_iters=0):
        concat_in = [
            np.concatenate([np.asarray(in_maps[c][nm]) for c in range(NCORES)],
                           axis=0)
            for nm in in_names
        ]
        def zeros():
            return [np.zeros((NCORES * z.shape[0], *z.shape[1:]), z.dtype)
                    for z in zero_outs]
        out_arrs = sharded(*concat_in, *zeros())
        out_arrs = [np.asarray(a) for a in out_arrs]
        results = [
            {nm: out_arrs[i].reshape(NCORES, *out_avals[i].shape)[c]
             for i, nm in enumerate(out_names)}
            for c in range(NCORES)
        ]
        times = []
        if timing_iters:
            import jax as _jax
            sh = _jax.sharding.NamedSharding(mesh, PartitionSpec("core"))
            dev_in = [_jax.device_put(a, sh) for a in concat_in]
            for _ in range(timing_iters):
                dz = [_jax.device_put(z, sh) for z in zeros()]
                _jax.block_until_ready(dz)
                t0 = time.perf_counter()
                o = sharded(*dev_in, *dz)
                _jax.block_until_ready(o)
                times.append(time.perf_counter() - t0)
        return results, times

    return run


def _get_runner():
    if "runner" not in _CACHED:
        _CACHED["runner"] = _make_runner(_get_program())
    return _CACHED["runner"]


def make_in_maps(prep, W1, b1, W2, b2, Wc1, bc1, Wc2, bc2, xi):
    W1 = np.asarray(W1, np.float32)
    b1 = np.asarray(b1, np.float32)
    W2 = np.asarray(W2, np.float32)
    b2 = np.asarray(b2, np.float32)
    Wc1 = np.asarray(Wc1, np.float32)
    bc1 = np.asarray(bc1, np.float32)
    Wc2 = np.asarray(Wc2, np.float32)
    bc2 = np.asarray(bc2, np.float32)
    xi = np.asarray(xi, np.float32)

    # batch/hop-masked packed weights (zero rows/cols place each batch)
    w1x = np.zeros((B * D, (HOPS + 1) * B * H), np.float32)
    for a in range(HOPS + 1):
        for b in range(B):
            off = (a * B + b) * H
            w1x[2 * b:2 * b + 2, off:off + H] = W1[2 * a:2 * a + 2, :]
    w2x = np.zeros((H, (HOPS + 1) * B * F), np.float32)
    for k in range(HOPS + 1):
        for b in range(B):
            off = (k * B + b) * F
            w2x[:, off + Q * b:off + Q * b + Q] = W2[k * H:(k + 1) * H, :]
    wc1xm = np.zeros((F, B * H), np.float32)
    for b in range(B):
        wc1xm[Q * b:Q * b + Q, H * b:H * b + H] = Wc1
    wc2xm = np.zeros((H, B * F), np.float32)
    for b in range(B):
        wc2xm[:, F * b + Q * b:F * b + Q * b + Q] = Wc2
    b2t = np.tile(b2, B).reshape(F, 1)
    bc2t = np.tile(bc2, B).reshape(F, 1)
    m3 = np.zeros((F, 3 * B), np.float32)
    for b in range(B):
        m3[Q * b:Q * b + Q, 3 * b + 0] = 1.0
        m3[Q * b:Q * b + Q, 3 * b + 1] = xi
        m3[Q * b:Q * b + Q, 3 * b + 2] = xi * xi
    id128 = np.eye(P, dtype=np.float32)

    common = dict(
        feat_tab=prep["feat_tab"],
        W1X=w1x, b1=b1.reshape(H, 1), W2X=w2x, b2t=b2t,
        Wc1X=wc1xm, bc1=bc1.reshape(H, 1), Wc2X=wc2xm, bc2t=bc2t,
        M3=m3, id128=id128,
    )
    in_maps = []
    for c in range(NCORES):
        cd = prep["cores"][c]
        m = dict(common)
        m["gidx"] = cd["idx16"]
        m["s_alpha"] = cd["s_alpha"]
        m["s_norm"] = cd["s_norm"]
        m["featloc"] = cd["featloc"]
        in_maps.append(m)
    return in_maps


def postprocess(prep, results):
    predictions = np.zeros((B, OUT_LEN, N, D), np.float32)
    vsq_sum = 0.0
    for c in range(NCORES):
        cd = prep["cores"][c]
        pr = results[c]["pred_out"]           # [12, NLOC, 8]
        po = pr[:, cd["pi"], :]               # [12, NPER, 8] in local-node order
        n0 = c * NPER
        for b in range(B):
            predictions[b, :, n0:n0 + NPER, 0] = po[:, :, 2 * b + 0]
            predictions[b, :, n0:n0 + NPER, 1] = po[:, :, 2 * b + 1]
        vsq = results[c]["vsq_out"].reshape(P, NG, B)
        # valid lanes: local slot s = i*128+p must be a used pi position
        used = np.zeros(NLOC, bool)
        used[cd["pi"]] = True
        mask = used.reshape(1, -1)  # [1, NLOC]
        um = np.zeros((P, NG), bool)
        for i in range(NG):
            um[:, i] = used[CH0[i]:CH0[i] + P]
        um[0:OVL, NG - 1] = False  # rows duplicated from chunk NG-2
        vsq_sum += float((vsq * um[:, :, None]).sum())
    vloss = np.float32(vsq_sum / (OUT_LEN * B * N))
    return predictions, vloss


def kernel(inputs, src, dst, edge_w, W1, b1, W2, b2, Wc1, bc1, Wc2, bc2, xi):
    prep = _preprocess(inputs, src, dst, edge_w)
    in_maps = make_in_maps(prep, W1, b1, W2, b2, Wc1, bc1, Wc2, bc2, xi)
    results, _ = _get_runner()(in_maps)
    return postprocess(prep, results)


def kernel_timed(inputs, src, dst, edge_w, W1, b1, W2, b2, Wc1, bc1, Wc2, bc2,
                 xi, timing_iters=10):
    """Like kernel() but also returns per-iteration device wall times (s)."""
    prep = _preprocess(inputs, src, dst, edge_w)
    in_maps = make_in_maps(prep, W1, b1, W2, b2, Wc1, bc1, Wc2, bc2, xi)
    results, times = _get_runner()(in_maps, timing_iters=timing_iters)
    return postprocess(prep, results), times
